# revision 10
# baseline (speedup 1.0000x reference)
"""Channel-attention kernel for Trainium2, data-parallel over batch on 8 NeuronCores.

Reference computation (per batch b):
    xr   = x[b].reshape(HW, C)                  # [4096, 512] fp32
    s    = xr^T @ xr                            # [C, C] gram matrix
    attn = softmax(s, axis=-1)
    v    = xr @ attn                            # [4096, 512]
    out  = beta * v + x[b]

Device strategy (per core: 2 batches, software-pipelined):
  - load x[b] fp32 natural layout (sync ring), cast to bf16 (ScalarE)
  - SWDGE cast-DMA (gpsimd) bounces the fp32 tiles to a DRAM scratch as bf16
    (depends only on the load); per (channel-block, quarter) a big
    DMA-transpose (scalar ring, emitted after all casts so it never blocks
    them) reads it back as the [C-part, HW-free] layout for GEMM2
  - GEMM1 (gram) on TensorE in bf16, nt-major head so PE advances in lockstep
    with the casts, cb-major tail so softmax starts before GEMM1 ends
  - softmax rows on DVE+ScalarE straight out of PSUM; beta is folded into the
    normalization (attn_scaled = beta * exp(s - max) / sum), so the epilogue
    is one PSUM+SBUF add and beta=0 gives bit-exact x
  - GEMM2 on TensorE in bf16 into 2-bank PSUM tiles, consuming xt quarters in
    order (so the next batch's transposes can reuse the slots early)
  - epilogue on DVE: out = v + x_fp32; output stores on the sync ring,
    interleaved (in emission) with the next batch's loads so neither blocks
    the other
"""

import numpy as np

import concourse.bass as bass
import concourse.tile as tile
from concourse import bacc, mybir
from concourse.bass_utils import run_bass_kernel_spmd

N_CORES = 8
B_FULL = 16
B_PER_CORE = B_FULL // N_CORES  # 2
H = 64
W = 64
HW = H * W  # 4096
C = 512
NT = HW // 128  # 32 row tiles
CB = C // 128  # 4 channel blocks
NQ = 4  # scratch quarters per batch
QNT = NT // NQ  # 8 row tiles per quarter
QROWS = QNT * 128  # 1024 rows per quarter

# row-tiles per load group (small first groups shorten the PE prologue);
# every group has even size so the 2-row-tile epilogue chunks stay in-group,
# and groups never straddle a quarter boundary (2+2+4 = 8 = one quarter)
GROUPS = [2, 2, 4, 4, 4, 4, 4, 4, 4]
G_NT0 = [sum(GROUPS[:i]) for i in range(len(GROUPS))]  # first nt of each group
NT2G = {}  # nt -> (group index, offset inside group)
for _gi, (_n0, _sz) in enumerate(zip(G_NT0, GROUPS)):
    for _k in range(_sz):
        NT2G[_n0 + _k] = (_gi, _k)

F32 = mybir.dt.float32
BF16 = mybir.dt.bfloat16
AXL = mybir.AxisListType
ALU = mybir.AluOpType
ACTFN = mybir.ActivationFunctionType


class BatchState:
    def __init__(self):
        self.xf = []  # fp32 group tiles
        self.xbf = []  # bf16 group tiles
        self.scr = {}  # quarter -> DRAM scratch tile
        self.xtq = {}  # (cb, q) -> [128, QROWS] bf16 tile
        self.s_ps = []
        self.attn = []


def emit_in_group(nc, pools, x_ap, b, gi, st):
    """Load group gi of batch b, cast it, and bounce it to the DRAM scratch."""
    sz = GROUPS[gi]
    nt0 = G_NT0[gi]
    q = nt0 // QNT
    if q not in st.scr:
        st.scr[q] = pools["scr"].tile(
            [QROWS, C], BF16, tag="scr", name=f"scr_b{b}_q{q}"
        )
    r0 = nt0 * 128
    t = pools["xf"].tile([128, sz * C], F32, tag="xf", name=f"xf_b{b}_g{gi}")
    nc.sync.dma_start(
        t[:, :].rearrange("p (f c) -> p f c", c=C),
        x_ap[b, r0 : r0 + sz * 128, :].rearrange("(f p) c -> p f c", p=128),
    )
    bf = pools["xbf"].tile([128, sz * C], BF16, tag="xbf", name=f"xbf_b{b}_g{gi}")
    nc.scalar.copy(bf[:, :], t[:, :])
    lr0 = (nt0 - q * QNT) * 128  # row offset inside the quarter scratch
    nc.gpsimd.dma_start(
        st.scr[q][lr0 : lr0 + sz * 128, :].rearrange("(f p) c -> p f c", p=128),
        t[:, :].rearrange("p (f c) -> p f c", c=C),
    )
    st.xf.append(t)
    st.xbf.append(bf)


def emit_transposes(nc, pools, b, st):
    """DMA-transpose each quarter's scratch into [C-part, QROWS] tiles."""
    for q in range(NQ):
        for cb in range(CB):
            xt = pools["xt"].tile(
                [128, QROWS], BF16, tag="xt", name=f"xt_b{b}_q{q}_c{cb}"
            )
            nc.scalar.dma_start(
                xt[:, :],
                st.scr[q][:, cb * 128 : (cb + 1) * 128],
                transpose=True,
            )
            st.xtq[(cb, q)] = xt


def emit_gemm1(nc, pools, b, st):
    st.s_ps = [
        pools["ps_s"].tile([128, C], F32, tag="s", name=f"s_b{b}_{cb}")
        for cb in range(CB)
    ]
    TAIL = 8

    def g1mm(nt, cb):
        gi, k = NT2G[nt]
        nc.tensor.matmul(
            st.s_ps[cb][:, :],
            st.xbf[gi][:, k * C + cb * 128 : k * C + (cb + 1) * 128],
            st.xbf[gi][:, k * C : (k + 1) * C],
            start=(nt == 0),
            stop=(nt == NT - 1),
        )

    for nt in range(NT - TAIL):
        for cb in range(CB):
            g1mm(nt, cb)
    for cb in range(CB):
        for nt in range(NT - TAIL, NT):
            g1mm(nt, cb)


def emit_softmax(nc, pools, beta_bc, b, st):
    for cb in range(CB):
        nmax = pools["st"].tile([128, 1], F32, tag="nmax")
        nc.vector.tensor_reduce(
            nmax[:, :], st.s_ps[cb][:, :], axis=AXL.X, op=ALU.max, negate=True
        )
        exps = pools["sm"].tile([128, C], BF16, tag="exps")
        ssum = pools["st"].tile([128, 1], F32, tag="ssum")
        nc.scalar.activation(
            exps[:, :],
            st.s_ps[cb][:, :],
            ACTFN.Exp,
            bias=nmax[:, :],
            scale=1.0,
            accum_out=ssum[:, :],
        )
        rinv = pools["st"].tile([128, 1], F32, tag="rinv")
        nc.vector.reciprocal(rinv[:, :], ssum[:, :])
        rsc = pools["st"].tile([128, 1], F32, tag="rsc")
        nc.vector.tensor_mul(rsc[:, :], rinv[:, :], beta_bc[:, :])
        at = pools["sm"].tile([128, C], BF16, tag="attn")
        nc.vector.tensor_scalar_mul(at[:, :], exps[:, :], rsc[:, :])
        st.attn.append(at)


def emit_g2_chunk(nc, pools, out_ap, b, np_, st):
    """GEMM2 + epilogue for row tiles (2*np_, 2*np_+1)."""
    vps = pools["ps_v"].tile([128, 2 * C], F32, tag="v")
    for j in range(2):
        nt = np_ * 2 + j
        q = nt // QNT
        for cb in range(CB):
            nc.tensor.matmul(
                vps[:, j * C : (j + 1) * C],
                st.xtq[(cb, q)][:, (nt - q * QNT) * 128 : (nt - q * QNT + 1) * 128],
                st.attn[cb][:, :],
                start=(cb == 0),
                stop=(cb == CB - 1),
            )
    ot = pools["outp"].tile([128, 2 * C], F32, tag="o")
    gi, k = NT2G[np_ * 2]
    nc.vector.tensor_add(
        ot[:, :], vps[:, :], st.xf[gi][:, k * C : (k + 2) * C]
    )
    nc.sync.dma_start(
        out_ap[b, np_ * 256 : (np_ + 1) * 256, :].rearrange(
            "(f p) c -> p f c", p=128
        ),
        ot[:, :].rearrange("p (f c) -> p f c", c=C),
    )


def channel_attention_body(tc, out_ap, x_ap, beta_ap):
    nc = tc.nc
    from contextlib import ExitStack

    with ExitStack() as ctx:
        ep = ctx.enter_context
        pools = {
            "xf": ep(tc.tile_pool(name="xf", bufs=11)),
            "xbf": ep(tc.tile_pool(name="xbf", bufs=9)),
            "xt": ep(tc.tile_pool(name="xt", bufs=20)),
            "sm": ep(tc.tile_pool(name="sm", bufs=5)),
            "st": ep(tc.tile_pool(name="st", bufs=8)),
            "outp": ep(tc.tile_pool(name="outp", bufs=4)),
            "const": ep(tc.tile_pool(name="const", bufs=1)),
            "scr": ep(tc.tile_pool(name="scr", bufs=6, space="DRAM")),
            "ps_s": ep(tc.tile_pool(name="ps_s", bufs=4, space="PSUM")),
            "ps_v": ep(tc.tile_pool(name="ps_v", bufs=2, space="PSUM")),
        }

        # beta -> broadcast to [128, 1]
        beta_sb = pools["const"].tile([1, 1], F32, tag="beta")
        nc.sync.dma_start(beta_sb[0:1, 0:1], beta_ap[None, :])
        beta_bc = pools["const"].tile([128, 1], F32, tag="beta_bc")
        nc.gpsimd.partition_broadcast(beta_bc[:, :], beta_sb[0:1, :])

        NGRP = len(GROUPS)
        states = [BatchState() for _ in range(B_PER_CORE)]

        # prologue: batch 0 input stage
        for gi in range(NGRP):
            emit_in_group(nc, pools, x_ap, 0, gi, states[0])
        emit_transposes(nc, pools, 0, states[0])

        for b in range(B_PER_CORE):
            emit_gemm1(nc, pools, b, states[b])
            emit_softmax(nc, pools, beta_bc, b, states[b])
            nxt = b + 1
            gi_next = 0
            for np_ in range(NT // 2):
                emit_g2_chunk(nc, pools, out_ap, b, np_, states[b])
                if nxt < B_PER_CORE and np_ % 2 == 1 and gi_next < NGRP:
                    emit_in_group(nc, pools, x_ap, nxt, gi_next, states[nxt])
                    gi_next += 1
            if nxt < B_PER_CORE:
                while gi_next < NGRP:
                    emit_in_group(nc, pools, x_ap, nxt, gi_next, states[nxt])
                    gi_next += 1
                emit_transposes(nc, pools, nxt, states[nxt])


_NC_CACHE = None


def _build():
    global _NC_CACHE
    if _NC_CACHE is not None:
        return _NC_CACHE
    nc = bacc.Bacc(
        "TRN2",
        target_bir_lowering=False,
        debug=False,
        num_devices=N_CORES,
    )
    x_ap = nc.dram_tensor("x", [B_PER_CORE, HW, C], F32, kind="ExternalInput").ap()
    beta_ap = nc.dram_tensor("beta", [1], F32, kind="ExternalInput").ap()
    out_ap = nc.dram_tensor(
        "out", [B_PER_CORE, HW, C], F32, kind="ExternalOutput"
    ).ap()
    with tile.TileContext(nc) as tc:
        channel_attention_body(tc, out_ap, x_ap, beta_ap)
    nc.compile()
    _NC_CACHE = nc
    return nc


def run(x, beta, trace=False, **trace_kwargs):
    """Shard over batch, run on 8 cores, gather. Returns (out, BassKernelResults)."""
    x = np.asarray(x, dtype=np.float32)
    beta = np.asarray(beta, dtype=np.float32)
    assert x.shape == (B_FULL, H, W, C), x.shape
    nc = _build()
    xr = x.reshape(B_FULL, HW, C)
    in_maps = [
        {
            "x": np.ascontiguousarray(
                xr[i * B_PER_CORE : (i + 1) * B_PER_CORE]
            ),
            "beta": beta,
        }
        for i in range(N_CORES)
    ]
    res = run_bass_kernel_spmd(
        nc, in_maps, core_ids=list(range(N_CORES)), trace=trace, **trace_kwargs
    )
    out = np.concatenate([res.results[i]["out"] for i in range(N_CORES)], axis=0)
    return out.reshape(B_FULL, H, W, C), res


def kernel(x, beta):
    out, _ = run(x, beta, trace=False)
    return out
